# revision 1
# baseline (speedup 1.0000x reference)
"""GaussianBlur2d Trainium2 kernel: 13x13 separable gaussian blur, reflect pad.

Input : x [32, 1, 1024, 1024] f32, kernel [1, 1, 13, 13] f32 (rank-1 separable).
Output: [32, 1, 1024, 1024] f32.

Strategy (pure data parallel, 4 images per core on 8 cores):
  The 2D conv is factored (SVD rank-1) into a vertical and a horizontal
  13-tap pass. Each pass runs on the TensorEngine as banded matmuls with
  an IMAGE TILE as the stationary operand:

     out[m=col, n=out_row] = sum_k  Xtile[k=row, m=col] * B[k=row, n=out_row]

  which both applies the 13-tap band (B) along the contraction (row) dim
  and transposes the tile - so after pass 1 the intermediate T1^T has
  partition=col, which is exactly the contraction layout pass 2 needs.

  Stationary tiles are 128-row windows at stride 116 (6-row halo on each
  side), so every 116-wide output block is fully computed by a single
  matmul - no cross-segment PSUM accumulation, no tiny spill matmuls.
  The first/last windows are image-aligned and the reflect-pad taps fold
  into their band matrices (reflected rows always land inside the
  clipped window). Output blocks are disjoint, so matmuls write
  disjoint PSUM column ranges of three packed PSUM tiles per group.
"""
import numpy as np

import concourse.bacc as bacc
import concourse.mybir as mybir
import concourse.tile as tile
from concourse import bass_utils

F32 = mybir.dt.float32

H = 1024          # image rows/cols
SEG = 128         # stationary window height (contraction K)
KS = 13
HALF = KS // 2
N_CORES = 8
IMGS_PER_CORE = 4

# output blocks: [0,122) from the aligned first window, then stride 116,
# last block [934,1024) from the aligned last window
BLOCK_STARTS = [0] + [122 + 116 * i for i in range(7)] + [934]
BLOCK_ENDS = [122] + [122 + 116 * (i + 1) for i in range(7)] + [1024]
NBLK = 9
# stationary window first row per block (clipped to the image)
WIN_STARTS = [0] + [122 + 116 * i - HALF for i in range(7)] + [H - SEG]
# psum packing: blocks 0-3 -> tile 0 (470 cols), 4-7 -> tile 1 (464), 8 -> tile 2 (90)
PSUM_OF_BLK = [0, 0, 0, 0, 1, 1, 1, 1, 2]
PSUM_WIDTH = [470, 464, 90]
PSUM_BASE = [0, 470, 934]  # column offset of each psum tile in the 1024 output
BAND_COLS = 1024


def _reflect(r):
    if r < 0:
        return -r
    if r > H - 1:
        return 2 * (H - 1) - r
    return r


def _decompose_kernel(k2d):
    k = np.asarray(k2d, dtype=np.float64).reshape(KS, KS)
    u, s, vh = np.linalg.svd(k)
    gv = u[:, 0] * np.sqrt(s[0])
    gh = vh[0, :] * np.sqrt(s[0])
    if gv.sum() < 0:
        gv, gh = -gv, -gh
    return gv, gh


def _plan():
    """Per-group MM plan: (blk, r0, o0, width, band_off, psum_idx, n0)."""
    plan = []
    off = 0
    for blk in range(NBLK):
        o0, o1 = BLOCK_STARTS[blk], BLOCK_ENDS[blk]
        r0 = WIN_STARTS[blk]
        p = PSUM_OF_BLK[blk]
        plan.append((blk, r0, o0, o1 - o0, off, p, o0 - PSUM_BASE[p]))
        off += o1 - o0
    assert off == BAND_COLS
    return plan


_PLAN = _plan()


def _build_bands(g):
    """Concatenated band matrices [128, 1024] f32 for one pass."""
    out = np.zeros((SEG, BAND_COLS), dtype=np.float64)
    for (blk, r0, o0, width, off, p, n0) in _PLAN:
        for n in range(width):
            for t in range(KS):
                rr = _reflect(o0 + n - HALF + t)
                if r0 <= rr < r0 + SEG:
                    out[rr - r0, off + n] += g[t]
    return out.astype(np.float32)


def _build_program(shared_bands):
    # shared_bands: separable factors equal (symmetric kernel) -> one band
    # array serves both passes, halving the critical-path bands DMA
    nbc = BAND_COLS if shared_bands else 2 * BAND_COLS
    p2off = 0 if shared_bands else BAND_COLS
    nc = bacc.Bacc("TRN2", target_bir_lowering=False, debug=False)
    x = nc.dram_tensor("x", [IMGS_PER_CORE, H, H], F32, kind="ExternalInput")
    bands = nc.dram_tensor("bands", [SEG, nbc], F32, kind="ExternalInput")
    y = nc.dram_tensor("y", [IMGS_PER_CORE, H, H], F32, kind="ExternalOutput")

    with tile.TileContext(nc) as tc:
        with (
            tc.tile_pool(name="xp", bufs=2) as xp,
            tc.tile_pool(name="t1p", bufs=1) as t1p,
            tc.tile_pool(name="op", bufs=2) as op,
            tc.tile_pool(name="bp", bufs=1) as bp,
            tc.tile_pool(name="ps", bufs=2, space="PSUM") as psp,
        ):
            bt = bp.tile([SEG, nbc], F32, tag="bands")
            nc.sync.dma_start(bt[:], bands[:])

            for b in range(IMGS_PER_CORE):
                # overlapping 128-row stationary windows (stride 116)
                xts = []
                for blk in range(NBLK):
                    r0 = WIN_STARTS[blk]
                    xs = xp.tile([SEG, H], F32, name=f"xt{blk}", tag=f"x{blk}")
                    nc.sync.dma_start(xs[:], x[b, r0:r0 + SEG, :])
                    xts.append(xs)
                t1 = t1p.tile([SEG, NBLK * H], F32, name="t1", tag="t1")
                # pass 1: vertical taps; col-group cg covers image cols
                # [WIN_STARTS[cg], +128); output T1^T group [col-local, row]
                for cg in range(NBLK):
                    c0 = WIN_STARTS[cg]
                    ps = [psp.tile([SEG, PSUM_WIDTH[i]], F32, name=f"psv{i}",
                                   tag=f"ps{i}", bufs=3 if i < 2 else 2) for i in range(3)]
                    done = set()
                    for (blk, r0, o0, width, off, p, n0) in _PLAN:
                        nc.tensor.matmul(
                            ps[p][:, n0:n0 + width],
                            xts[blk][:, c0:c0 + SEG],
                            bt[:, off:off + width],
                            start=(p not in done), stop=(blk in (3, 7, 8)),
                        )
                        done.add(p)
                    for i in range(3):
                        nc.vector.tensor_copy(
                            t1[:, cg * H + PSUM_BASE[i]: cg * H + PSUM_BASE[i] + PSUM_WIDTH[i]],
                            ps[i][:],
                        )
                # pass 2: horizontal taps on T1^T; row-group j covers out
                # rows [128j, 128j+128); stationaries are t1 col-groups
                for j in range(8):
                    ps = [psp.tile([SEG, PSUM_WIDTH[i]], F32, name=f"psh{i}",
                                   tag=f"ps{i}", bufs=3 if i < 2 else 2) for i in range(3)]
                    done = set()
                    for (blk, r0, o0, width, off, p, n0) in _PLAN:
                        nc.tensor.matmul(
                            ps[p][:, n0:n0 + width],
                            t1[:, blk * H + j * SEG: blk * H + j * SEG + SEG],
                            bt[:, p2off + off: p2off + off + width],
                            start=(p not in done), stop=(blk in (3, 7, 8)),
                        )
                        done.add(p)
                    oj = op.tile([SEG, H], F32, name=f"ot{j}", tag=f"o{j % 4}")
                    for i in range(3):
                        nc.scalar.copy(
                            oj[:, PSUM_BASE[i]: PSUM_BASE[i] + PSUM_WIDTH[i]],
                            ps[i][:],
                        )
                    nc.sync.dma_start(y[b, j * SEG:(j + 1) * SEG, :], oj[:])
    nc.compile()
    return nc


_NC_CACHE = {}


def _get_program(shared_bands):
    if shared_bands not in _NC_CACHE:
        _NC_CACHE[shared_bands] = _build_program(shared_bands)
    return _NC_CACHE[shared_bands]


def run(x, kernel, trace=False, tmpdir=None):
    """Full-input entry. Returns (y, BassKernelResults)."""
    x = np.ascontiguousarray(np.asarray(x, dtype=np.float32).reshape(32, H, H))
    gv, gh = _decompose_kernel(kernel)
    shared = bool(np.allclose(gv, gh, rtol=0, atol=1e-12 * np.abs(gv).max()))
    if shared:
        bands = _build_bands(gv)
    else:
        bands = np.concatenate([_build_bands(gv), _build_bands(gh)], axis=1)
    nc = _get_program(shared)
    in_maps = [
        {"x": x[c * IMGS_PER_CORE:(c + 1) * IMGS_PER_CORE], "bands": bands}
        for c in range(N_CORES)
    ]
    res = bass_utils.run_bass_kernel_spmd(
        nc, in_maps, core_ids=list(range(N_CORES)), trace=trace, tmpdir=tmpdir)
    y = np.concatenate([res.results[c]["y"] for c in range(N_CORES)], axis=0)
    return y.reshape(32, 1, H, H), res


def kernel(x, kernel):
    y, _ = run(x, kernel, trace=False)
    return y



# revision 2
# speedup vs baseline: 1.6305x; 1.6305x over previous
"""GaussianBlur2d Trainium2 kernel: 13x13 separable gaussian blur, reflect pad.

Input : x [32, 1, 1024, 1024] f32, kernel [1, 1, 13, 13] f32 (rank-1 separable).
Output: [32, 1, 1024, 1024] f32.

Strategy (pure data parallel, 4 images per core on 8 cores):
  The 2D conv is factored (SVD rank-1) into a vertical and a horizontal
  13-tap pass. Each pass runs on the TensorEngine as banded matmuls with
  an IMAGE TILE as the stationary operand:

     out[m=col, n=out_row] = sum_k  Xtile[k=row, m=col] * B[k=row, n=out_row]

  which both applies the 13-tap band (B) along the contraction (row) dim
  and transposes the tile - so after pass 1 the intermediate T1^T has
  partition=col, which is exactly the contraction layout pass 2 needs.

  Stationary tiles are 128-row windows at stride 116 (6-row halo on each
  side), so every 116-wide output block is fully computed by a single
  matmul - no cross-segment PSUM accumulation, no tiny spill matmuls.
  The first/last windows are image-aligned and the reflect-pad taps fold
  into their band matrices (reflected rows always land inside the
  clipped window). Output blocks are disjoint, so matmuls write
  disjoint PSUM column ranges of three packed PSUM tiles per group.

  The whole datapath runs in bf16 (inputs, bands, intermediate, output;
  PSUM accumulation stays fp32): fp32 matmuls on trn2 lower to
  FP32HI/FP32LO pairs (2x PE work) and disable Fast Weight Load, and
  fp32 I/O doubles HBM traffic. bf16 keeps rel-err ~1e-3, far inside
  the 2e-2 gate.
"""
import numpy as np
import ml_dtypes

import concourse.bacc as bacc
import concourse.mybir as mybir
import concourse.tile as tile
from concourse import bass_utils

F32 = mybir.dt.float32
BF16 = mybir.dt.bfloat16
NP_BF16 = ml_dtypes.bfloat16

H = 1024          # image rows/cols
SEG = 128         # stationary window height (contraction K)
KS = 13
HALF = KS // 2
N_CORES = 8
IMGS_PER_CORE = 4

# output blocks: [0,122) from the aligned first window, then stride 116,
# last block [934,1024) from the aligned last window
BLOCK_STARTS = [0] + [122 + 116 * i for i in range(7)] + [934]
BLOCK_ENDS = [122] + [122 + 116 * (i + 1) for i in range(7)] + [1024]
NBLK = 9
# stationary window first row per block (clipped to the image)
WIN_STARTS = [0] + [122 + 116 * i - HALF for i in range(7)] + [H - SEG]
# psum packing: blocks 0-3 -> tile 0 (470 cols), 4-7 -> tile 1 (464), 8 -> tile 2 (90)
PSUM_OF_BLK = [0, 0, 0, 0, 1, 1, 1, 1, 2]
PSUM_WIDTH = [470, 464, 90]
PSUM_BASE = [0, 470, 934]  # column offset of each psum tile in the 1024 output
BAND_COLS = 1024


def _reflect(r):
    if r < 0:
        return -r
    if r > H - 1:
        return 2 * (H - 1) - r
    return r


def _decompose_kernel(k2d):
    k = np.asarray(k2d, dtype=np.float64).reshape(KS, KS)
    u, s, vh = np.linalg.svd(k)
    gv = u[:, 0] * np.sqrt(s[0])
    gh = vh[0, :] * np.sqrt(s[0])
    if gv.sum() < 0:
        gv, gh = -gv, -gh
    return gv, gh


def _plan():
    """Per-group MM plan: (blk, r0, o0, width, band_off, psum_idx, n0)."""
    plan = []
    off = 0
    for blk in range(NBLK):
        o0, o1 = BLOCK_STARTS[blk], BLOCK_ENDS[blk]
        r0 = WIN_STARTS[blk]
        p = PSUM_OF_BLK[blk]
        plan.append((blk, r0, o0, o1 - o0, off, p, o0 - PSUM_BASE[p]))
        off += o1 - o0
    assert off == BAND_COLS
    return plan


_PLAN = _plan()


def _build_bands(g):
    """Concatenated band matrices [128, 1024] for one pass."""
    out = np.zeros((SEG, BAND_COLS), dtype=np.float64)
    for (blk, r0, o0, width, off, p, n0) in _PLAN:
        for n in range(width):
            for t in range(KS):
                rr = _reflect(o0 + n - HALF + t)
                if r0 <= rr < r0 + SEG:
                    out[rr - r0, off + n] += g[t]
    return out.astype(NP_BF16)


def _build_program(shared_bands):
    # shared_bands: separable factors equal (symmetric kernel) -> one band
    # array serves both passes, halving the critical-path bands DMA
    nbc = BAND_COLS if shared_bands else 2 * BAND_COLS
    p2off = 0 if shared_bands else BAND_COLS
    nc = bacc.Bacc("TRN2", target_bir_lowering=False, debug=False)
    x = nc.dram_tensor("x", [IMGS_PER_CORE, H, H], BF16, kind="ExternalInput")
    bands = nc.dram_tensor("bands", [SEG, nbc], BF16, kind="ExternalInput")
    y = nc.dram_tensor("y", [IMGS_PER_CORE, H, H], BF16, kind="ExternalOutput")

    with tile.TileContext(nc) as tc:
        with (
            tc.tile_pool(name="xp", bufs=2) as xp,
            tc.tile_pool(name="t1p", bufs=1) as t1p,
            tc.tile_pool(name="op", bufs=2) as op,
            tc.tile_pool(name="bp", bufs=1) as bp,
            tc.tile_pool(name="ps", bufs=2, space="PSUM") as psp,
        ):
            bt = bp.tile([SEG, nbc], BF16, tag="bands")
            nc.sync.dma_start(bt[:], bands[:])

            for b in range(IMGS_PER_CORE):
                # overlapping 128-row stationary windows (stride 116)
                xts = []
                for blk in range(NBLK):
                    r0 = WIN_STARTS[blk]
                    xs = xp.tile([SEG, H], BF16, name=f"xt{blk}", tag=f"x{blk}")
                    nc.sync.dma_start(xs[:], x[b, r0:r0 + SEG, :])
                    xts.append(xs)
                t1 = t1p.tile([SEG, NBLK * H], BF16, name="t1", tag="t1")
                # pass 1: vertical taps; col-group cg covers image cols
                # [WIN_STARTS[cg], +128); output T1^T group [col-local, row]
                for cg in range(NBLK):
                    c0 = WIN_STARTS[cg]
                    ps = [psp.tile([SEG, PSUM_WIDTH[i]], F32, name=f"psv{i}",
                                   tag=f"ps{i}", bufs=3 if i < 2 else 2) for i in range(3)]
                    done = set()
                    for (blk, r0, o0, width, off, p, n0) in _PLAN:
                        nc.tensor.matmul(
                            ps[p][:, n0:n0 + width],
                            xts[blk][:, c0:c0 + SEG],
                            bt[:, off:off + width],
                            start=(p not in done), stop=(blk in (3, 7, 8)),
                        )
                        done.add(p)
                    for i in range(3):
                        nc.vector.tensor_copy(
                            t1[:, cg * H + PSUM_BASE[i]: cg * H + PSUM_BASE[i] + PSUM_WIDTH[i]],
                            ps[i][:],
                        )
                # pass 2: horizontal taps on T1^T; row-group j covers out
                # rows [128j, 128j+128); stationaries are t1 col-groups
                for j in range(8):
                    ps = [psp.tile([SEG, PSUM_WIDTH[i]], F32, name=f"psh{i}",
                                   tag=f"ps{i}", bufs=3 if i < 2 else 2) for i in range(3)]
                    done = set()
                    for (blk, r0, o0, width, off, p, n0) in _PLAN:
                        nc.tensor.matmul(
                            ps[p][:, n0:n0 + width],
                            t1[:, blk * H + j * SEG: blk * H + j * SEG + SEG],
                            bt[:, p2off + off: p2off + off + width],
                            start=(p not in done), stop=(blk in (3, 7, 8)),
                        )
                        done.add(p)
                    oj = op.tile([SEG, H], BF16, name=f"ot{j}", tag=f"o{j % 4}")
                    for i in range(3):
                        nc.scalar.copy(
                            oj[:, PSUM_BASE[i]: PSUM_BASE[i] + PSUM_WIDTH[i]],
                            ps[i][:],
                        )
                    nc.sync.dma_start(y[b, j * SEG:(j + 1) * SEG, :], oj[:])
    nc.compile()
    return nc


_NC_CACHE = {}


def _get_program(shared_bands):
    if shared_bands not in _NC_CACHE:
        _NC_CACHE[shared_bands] = _build_program(shared_bands)
    return _NC_CACHE[shared_bands]


def run(x, kernel, trace=False, tmpdir=None):
    """Full-input entry. Returns (y, BassKernelResults)."""
    x = np.ascontiguousarray(
        np.asarray(x, dtype=np.float32).reshape(32, H, H)).astype(NP_BF16)
    gv, gh = _decompose_kernel(kernel)
    shared = bool(np.allclose(gv, gh, rtol=0, atol=1e-12 * np.abs(gv).max()))
    if shared:
        bands = _build_bands(gv)
    else:
        bands = np.concatenate([_build_bands(gv), _build_bands(gh)], axis=1)
    nc = _get_program(shared)
    in_maps = [
        {"x": x[c * IMGS_PER_CORE:(c + 1) * IMGS_PER_CORE], "bands": bands}
        for c in range(N_CORES)
    ]
    res = bass_utils.run_bass_kernel_spmd(
        nc, in_maps, core_ids=list(range(N_CORES)), trace=trace, tmpdir=tmpdir)
    y = np.concatenate([res.results[c]["y"] for c in range(N_CORES)], axis=0)
    return y.reshape(32, 1, H, H).astype(np.float32), res


def kernel(x, kernel):
    y, _ = run(x, kernel, trace=False)
    return y
